# revision 18
# baseline (speedup 1.0000x reference)
"""Trainium2 Bass kernel for EnergyIrrepModulation.

Computes out[m, e, d] = x[m, d] * gates_full[e, d] where
gates = MLP(e_feat) : [nE, n_copies], expanded to [nE, D] via the static
irrep index map for IRREPS = [(64, 1), (32, 3), (16, 5)].

Sharding: data-parallel over M (4096 rows -> 512 rows per core, 8 cores).
Gates/MLP params are replicated; each core redundantly computes the tiny MLP.

Per-core device plan:
  1. All MLP params arrive packed in ONE [128, 1080] tensor (single DMA);
     host pre-transposes e_feat so no on-device transposes are needed.
  2. Tiny MLP on the tensor engine; biases+ReLU fused on the scalar engine.
  3. Raw gates [100, 112] are flattened per e-chunk onto partition 0, then
     GPSIMD partition_broadcast replicates them to 128 partitions; the
     scalar engine expands 112 channels -> 240 dims per the irrep map.
  4. Main loop: stream x tiles [128, 240], multiply against the broadcast
     gates on the vector engine (stride-0 read of x over the e axis), and
     store [128, 3000] halves on both HWDGE rings (sync + scalar). This is
     HBM-write-bound: ~49 MB per core at ~358 GB/s.
"""

import sys
from contextlib import ExitStack

import numpy as np

try:
    import concourse.bass as bass  # noqa: F401
except ImportError:  # pragma: no cover
    sys.path.insert(0, "/opt/trn_rl_repo")
    import concourse.bass as bass

import concourse.bacc as bacc
import concourse.tile as tile
from concourse import mybir
from concourse.bass_utils import run_bass_kernel_spmd

FP32 = mybir.dt.float32

M, D = 4096, 240
NE, E_DIM, HIDDEN, NCOP = 100, 64, 256, 112
N_CORES = 8
MC = M // N_CORES          # 512 rows per core
MT = MC // 128             # 4 m-tiles of 128 rows
EC = 25                    # e-chunk size
NEC = NE // EC             # 4 e-chunks
CHUNK = EC * D             # 6000 out elements per chunk per partition
RCHUNK = EC * NCOP         # 2800 raw gate elements per chunk

# packed param layout (columns of the [128, NPARAM] tensor)
C_W2A, C_W2B = 0, 256
C_W3A, C_W3B = 512, 624
C_B1, C_B2 = 736, 738
C_W1 = 740                 # [64, 128] x 2 stacked on partition halves
C_ET = 868                 # e_featT [64, 100] duplicated on both halves
C_B3 = 968                 # [1, 112] on partition 0
NPARAM = 1080

_CACHE = {}


def _build_program():
    nc = bacc.Bacc(None, target_bir_lowering=False, debug=False)

    x_d = nc.dram_tensor("x", [MC, D], FP32, kind="ExternalInput")
    p_d = nc.dram_tensor("params", [128, NPARAM], FP32, kind="ExternalInput")
    out_d = nc.dram_tensor("out", [MC, NE * D], FP32, kind="ExternalOutput")

    with tile.TileContext(nc) as tc, ExitStack() as ctx:
        const_pool = ctx.enter_context(tc.tile_pool(name="const", bufs=1))
        mlp_pool = ctx.enter_context(tc.tile_pool(name="mlp", bufs=1))
        psum_mlp = ctx.enter_context(
            tc.tile_pool(name="psum_mlp", bufs=2, space="PSUM")
        )
        raw_pool = ctx.enter_context(tc.tile_pool(name="raw", bufs=4))
        x_pool = ctx.enter_context(tc.tile_pool(name="xin", bufs=2))
        out_pool = ctx.enter_context(tc.tile_pool(name="out", bufs=3))

        p_t = const_pool.tile([128, NPARAM], FP32)
        # critical first-layer params (W1, eT, biases) land first
        nc.sync.dma_start(out=p_t[:, C_B1:NPARAM], in_=p_d[:, C_B1:NPARAM])
        nc.scalar.dma_start(out=p_t[:, 0:C_B1], in_=p_d[:, 0:C_B1])
        ones_t = const_pool.tile([1, NE], FP32)
        nc.vector.memset(ones_t[:], 1.0)

        relu = mybir.ActivationFunctionType.Relu

        # ---- MLP: h1T = relu(W1^T e_featT + b1), two [128, 100] tiles ----
        h1T = []
        for c in range(2):
            pl, ph = 64 * c, 64 * (c + 1)
            ps = psum_mlp.tile([128, NE], FP32)
            nc.tensor.matmul(
                ps[:], p_t[pl:ph, C_W1 : C_W1 + 128], p_t[pl:ph, C_ET : C_ET + NE],
                start=True, stop=True,
            )
            h = mlp_pool.tile([128, NE], FP32, tag=f"h1T{c}")
            nc.scalar.activation(h[:], ps[:], relu, bias=p_t[:, C_B1 + c : C_B1 + c + 1])
            h1T.append(h)

        # ---- h2T = relu(W2^T h1T + b2) ----
        h2T = []
        for c in range(2):
            ps = psum_mlp.tile([128, NE], FP32)
            nc.tensor.matmul(
                ps[:], p_t[:, C_W2A + c * 128 : C_W2A + (c + 1) * 128], h1T[0][:],
                start=True, stop=False,
            )
            nc.tensor.matmul(
                ps[:], p_t[:, C_W2B + c * 128 : C_W2B + (c + 1) * 128], h1T[1][:],
                start=False, stop=True,
            )
            h = mlp_pool.tile([128, NE], FP32, tag=f"h2T{c}")
            nc.scalar.activation(h[:], ps[:], relu, bias=p_t[:, C_B2 + c : C_B2 + c + 1])
            h2T.append(h)

        # ---- gates = h2 @ W3 + b3 : psum [100, 112], partition = e ----
        psg = psum_mlp.tile([NE, NCOP], FP32)
        nc.tensor.matmul(
            psg[:], h2T[0][:], p_t[:, C_W3A : C_W3A + NCOP], start=True, stop=False
        )
        nc.tensor.matmul(
            psg[:], h2T[1][:], p_t[:, C_W3B : C_W3B + NCOP], start=False, stop=False
        )
        # += ones[100,1] @ b3[1,112]: bias along the free dim via PE
        nc.tensor.matmul(
            psg[:], ones_t[:], p_t[0:1, C_B3 : C_B3 + NCOP], start=False, stop=True
        )
        gates_t = mlp_pool.tile([NE, NCOP], FP32)
        nc.scalar.copy(gates_t[:], psg[:])

        # ---- broadcast gates to all 128 partitions ----
        # Chunk 0 takes the low-latency path: flatten onto partition 0 and
        # GPSIMD partition_broadcast (finishes before the first multiply, so
        # no SBUF-port contention with the vector engine). Chunks 1-3 go via
        # a DRAM bounce with stride-0 source reads on the then-idle DMA
        # engines; they complete long before their consumers.
        raws = []
        raw0 = raw_pool.tile([128, RCHUNK], FP32)
        st = mlp_pool.tile([1, RCHUNK], FP32)
        st_i = nc.sync.dma_start(out=st[:], in_=gates_t[0:EC, :])
        nc.gpsimd.partition_broadcast(raw0[:], st[0:1, :])
        raws.append(raw0)

        g_dram = nc.dram_tensor("gates_scratch", [NE * NCOP], FP32)
        bounce_i = nc.sync.dma_start(
            out=g_dram[RCHUNK:], in_=gates_t[EC:NE, :]
        )
        # keep the 16 SDMA engines free for the chunk-0 stage transfer:
        # the bulk bounce/broadcast reads have slack until ~mid-loop
        tile.add_dep_helper(
            bounce_i.ins, st_i.ins, sync=True,
            reason="defer bulk gate bcast behind chunk-0 stage",
        )
        for ec in range(1, NEC):
            raw = raw_pool.tile([128, RCHUNK], FP32)
            src = (
                g_dram[ec * RCHUNK : (ec + 1) * RCHUNK]
                .unsqueeze(0)
                .to_broadcast((128, RCHUNK))
            )
            eng = nc.sync if ec % 2 == 0 else nc.scalar
            eng.dma_start(out=raw[:], in_=src)
            raws.append(raw)

        # ---- main loop: out[m, e, d] = x[m, d] * gates[e, c(d)] ----
        # The irrep expansion (112 channels -> 240 dims) is fused into the
        # multiply via broadcast access patterns on the gate operand.
        half = CHUNK // 2
        for mt in range(MT):
            x_t = x_pool.tile([128, D], FP32)
            nc.gpsimd.dma_start(out=x_t[:], in_=x_d[mt * 128 : (mt + 1) * 128, :])
            x_v = x_t[:].unsqueeze(1).to_broadcast((128, EC, D))
            for ec in range(NEC):
                g_v = raws[ec][:].rearrange("p (e c) -> p e c", c=NCOP)
                o_t = out_pool.tile([128, CHUNK], FP32)
                o_v = o_t[:].rearrange("p (e d) -> p e d", d=D)
                nc.vector.tensor_mul(
                    o_v[:, :, 0:64], x_v[:, :, 0:64], g_v[:, :, 0:64]
                )
                nc.vector.tensor_mul(
                    o_v[:, :, 64:160].rearrange("p e (i k) -> p e i k", k=3),
                    x_v[:, :, 64:160].rearrange("p e (i k) -> p e i k", k=3),
                    g_v[:, :, 64:96].unsqueeze(3).to_broadcast((128, EC, 32, 3)),
                )
                nc.vector.tensor_mul(
                    o_v[:, :, 160:240].rearrange("p e (i k) -> p e i k", k=5),
                    x_v[:, :, 160:240].rearrange("p e (i k) -> p e i k", k=5),
                    g_v[:, :, 96:112].unsqueeze(3).to_broadcast((128, EC, 16, 5)),
                )
                # split the store across both HWDGE rings (SP + ACT)
                base = ec * CHUNK
                nc.sync.dma_start(
                    out=out_d[mt * 128 : (mt + 1) * 128, base : base + half],
                    in_=o_t[:, 0:half],
                )
                nc.scalar.dma_start(
                    out=out_d[mt * 128 : (mt + 1) * 128, base + half : base + CHUNK],
                    in_=o_t[:, half:CHUNK],
                )

    nc.compile()
    return nc


def _marshal(inputs):
    f32 = lambda a: np.ascontiguousarray(np.asarray(a, dtype=np.float32))
    x = f32(inputs["x"])
    W1, W2, W3 = f32(inputs["W1"]), f32(inputs["W2"]), f32(inputs["W3"])
    b1, b2, b3 = f32(inputs["b1"]), f32(inputs["b2"]), f32(inputs["b3"])
    eT = f32(np.asarray(inputs["e_feat"]).T)

    p = np.zeros((128, NPARAM), np.float32)
    p[:, C_W2A : C_W2A + 256] = W2[0:128]
    p[:, C_W2B : C_W2B + 256] = W2[128:256]
    p[:, C_W3A : C_W3A + NCOP] = W3[0:128]
    p[:, C_W3B : C_W3B + NCOP] = W3[128:256]
    p[:, C_B1] = b1[0:128]
    p[:, C_B1 + 1] = b1[128:256]
    p[:, C_B2] = b2[0:128]
    p[:, C_B2 + 1] = b2[128:256]
    p[0:64, C_W1 : C_W1 + 128] = W1[:, 0:128]
    p[64:128, C_W1 : C_W1 + 128] = W1[:, 128:256]
    p[0:64, C_ET : C_ET + NE] = eT
    p[64:128, C_ET : C_ET + NE] = eT
    p[0, C_B3 : C_B3 + NCOP] = b3

    return [
        {"x": x[i * MC : (i + 1) * MC], "params": p} for i in range(N_CORES)
    ]


def get_program():
    if "nc" not in _CACHE:
        _CACHE["nc"] = _build_program()
    return _CACHE["nc"]


def run(inputs, trace=False, **kwargs):
    """Run on 8 cores; returns (out [M, NE, D], BassKernelResults)."""
    nc = get_program()
    in_maps = _marshal(inputs)
    res = run_bass_kernel_spmd(
        nc, in_maps, core_ids=list(range(N_CORES)), trace=trace, **kwargs
    )
    out = np.concatenate(
        [np.asarray(res.results[i]["out"]).reshape(MC, NE, D) for i in range(N_CORES)],
        axis=0,
    )
    return out, res


def kernel(**inputs) -> np.ndarray:
    out, _ = run(inputs)
    return out
